# revision 3
# baseline (speedup 1.0000x reference)
"""Trainium2 Bass kernel for nn_Pointnet2DetHead (segment_reduce).

Pipeline per core (N sharded 8 ways, 12544 points/core in 98 chunks of 128):
  - box-membership mask[n,p] via exact fp32 compares:
      x-dim on DVE (tensor_scalar is_le/is_ge -> {0,2} int16)
      y/z dims on ACT (Sign(coord - lo), Sign(hi - coord) -> {-1,0,1} int16)
      combined with int16 scalar_tensor_tensor adds + threshold >= 7
  - sums[p,r] and counts[p] via float32r matmuls accumulated in PSUM
  - AllReduce partials across the 8 cores
  - replicated head: mean -> linear heads -> softmax(cls, axis=c) *
    softmax(obj, axis=p)
"""

import sys

if "/opt/trn_rl_repo" not in sys.path:
    sys.path.insert(0, "/opt/trn_rl_repo")

import numpy as np
from contextlib import ExitStack

import concourse.bass as bass
import concourse.tile as tile
import concourse.mybir as mybir
from concourse import bacc
from concourse.bass_utils import run_bass_kernel_spmd
from concourse.mybir import AluOpType as Op, ActivationFunctionType as Act, AxisListType

N_CORES = 8
N, P, R, CO = 100000, 256, 256, 21  # CO = C+1 output classes
CHUNK = 128
N_PAD_CORE = 12544            # 98 chunks of 128
N_CHUNKS = N_PAD_CORE // CHUNK
GROUP = 7                     # chunks per feats DMA group
N_GROUPS = N_CHUNKS // GROUP
FP = mybir.dt.float32
FPR = mybir.dt.float32r
I16 = mybir.dt.int16

_cache = {}


def _build():
    nc = bacc.Bacc("TRN2", num_devices=N_CORES, target_bir_lowering=False, debug=False)

    feats_d = nc.dram_tensor("feats", [CHUNK, N_CHUNKS * R], FPR, kind="ExternalInput")
    xyz_d = nc.dram_tensor("xyz", [CHUNK, N_CHUNKS * 3], FP, kind="ExternalInput")
    bt_d = nc.dram_tensor("btiles", [CHUNK, 6 * P], FP, kind="ExternalInput")
    wcat_d = nc.dram_tensor("wcat", [2 * CHUNK, 64], FPR, kind="ExternalInput")
    bcat_d = nc.dram_tensor("bcat", [64, 1], FP, kind="ExternalInput")
    ident_d = nc.dram_tensor("ident", [CHUNK, CHUNK], FP, kind="ExternalInput")
    ones_d = nc.dram_tensor("ones", [CHUNK, 1], FPR, kind="ExternalInput")
    c16_d = nc.dram_tensor("c16", [CHUNK, 4], I16, kind="ExternalInput")
    out_d = nc.dram_tensor("out", [P, CO], FP, kind="ExternalOutput")

    with ExitStack() as ctx:
        tc = ctx.enter_context(tile.TileContext(nc))
        const = ctx.enter_context(tc.tile_pool(name="const", bufs=1))
        fpool = ctx.enter_context(tc.tile_pool(name="fpool", bufs=3))
        cmp_p = ctx.enter_context(tc.tile_pool(name="cmp", bufs=3))
        mpool = ctx.enter_context(tc.tile_pool(name="mp", bufs=3))
        spool = ctx.enter_context(tc.tile_pool(name="sp", bufs=2))
        psA = ctx.enter_context(tc.tile_pool(name="psA", bufs=1, space="PSUM"))
        psB = ctx.enter_context(tc.tile_pool(name="psB", bufs=1, space="PSUM"))
        psS = ctx.enter_context(tc.tile_pool(name="psS", bufs=1, space="PSUM"))
        dram = ctx.enter_context(tc.tile_pool(name="dram", bufs=1, space="DRAM"))

        # ---- constants ----
        bt = const.tile([CHUNK, 6 * P], FP)       # Lx|Hx|Ly|Hy|Lz|Hz broadcast tiles
        nc.gpsimd.dma_start(bt[:], bt_d.ap()[:])
        xyz = const.tile([CHUNK, N_CHUNKS * 3], FP)
        nc.gpsimd.dma_start(xyz[:], xyz_d.ap()[:])
        wc0 = const.tile([CHUNK, 64], FPR)
        wc1 = const.tile([CHUNK, 64], FPR)
        nc.gpsimd.dma_start(wc0[:], wcat_d.ap()[0:CHUNK, :])
        nc.gpsimd.dma_start(wc1[:], wcat_d.ap()[CHUNK : 2 * CHUNK, :])
        bcat = const.tile([64, 1], FP)
        nc.gpsimd.dma_start(bcat[:], bcat_d.ap()[:])
        ident = const.tile([CHUNK, CHUNK], FP)
        nc.gpsimd.dma_start(ident[:], ident_d.ap()[:])
        ones = const.tile([CHUNK, 1], FPR)
        nc.gpsimd.dma_start(ones[:], ones_d.ap()[:])
        c16 = const.tile([CHUNK, 4], I16)
        nc.gpsimd.dma_start(c16[:], c16_d.ap()[:])
        m7 = c16[:, 0:1]   # -7
        m1 = c16[:, 1:2]   # -1
        z16 = c16[:, 2:3]  # 0

        nxyz = const.tile([CHUNK, N_CHUNKS * 3], FP)
        nc.vector.tensor_scalar(nxyz[:], xyz[:], -1.0, None, Op.mult)

        Lx, Hx = bt[:, 0:P], bt[:, P : 2 * P]
        Ly, Hy = bt[:, 2 * P : 3 * P], bt[:, 3 * P : 4 * P]
        Lz, Hz = bt[:, 4 * P : 5 * P], bt[:, 5 * P : 6 * P]

        # ---- accumulators ----
        ps_s0 = psA.tile([CHUNK, R], FP)   # sums for proposals 0..127
        ps_s1 = psA.tile([CHUNK, R], FP)   # sums for proposals 128..255
        ps_cnt = psA.tile([1, P], FP)      # counts

        # ---- main loop ----
        feats_g = None
        for i in range(N_CHUNKS):
            g, j = divmod(i, GROUP)
            if j == 0:
                feats_g = fpool.tile([CHUNK, GROUP * R], FPR, tag="feats")
                nc.gpsimd.dma_start(
                    feats_g[:], feats_d.ap()[:, g * GROUP * R : (g + 1) * GROUP * R]
                )
            xc = xyz[:, 3 * i : 3 * i + 1]
            yc = xyz[:, 3 * i + 1 : 3 * i + 2]
            zc = xyz[:, 3 * i + 2 : 3 * i + 3]
            nyc = nxyz[:, 3 * i + 1 : 3 * i + 2]
            nzc = nxyz[:, 3 * i + 2 : 3 * i + 3]

            A3 = cmp_p.tile([CHUNK, 3 * P], I16, tag="A3")
            B3 = cmp_p.tile([CHUNK, 3 * P], I16, tag="B3")
            # x-dim bools scaled to {0,2} on DVE
            nc.vector.tensor_scalar(A3[:, 0:P], Lx, xc, 2.0, Op.is_le, Op.mult)
            nc.vector.tensor_scalar(B3[:, 0:P], Hx, xc, 2.0, Op.is_ge, Op.mult)
            # y/z sign tests on ACT: {-1,0,1}
            nc.scalar.activation(A3[:, P : 2 * P], Ly, Act.Sign, bias=yc, scale=-1.0)
            nc.scalar.activation(B3[:, P : 2 * P], Hy, Act.Sign, bias=nyc, scale=1.0)
            nc.scalar.activation(A3[:, 2 * P : 3 * P], Lz, Act.Sign, bias=zc, scale=-1.0)
            nc.scalar.activation(B3[:, 2 * P : 3 * P], Hz, Act.Sign, bias=nzc, scale=1.0)

            U = cmp_p.tile([CHUNK, 3 * P], I16, tag="U")
            nc.vector.scalar_tensor_tensor(U[:], A3[:], z16, B3[:], Op.add, Op.add)
            Sp = cmp_p.tile([CHUNK, P], I16, tag="Sp")
            nc.vector.scalar_tensor_tensor(
                Sp[:], U[:, 0:P], m7, U[:, P : 2 * P], Op.add, Op.add
            )
            M = mpool.tile([CHUNK, P], FPR, tag="M")
            nc.vector.scalar_tensor_tensor(
                M[:], Sp[:], m1, U[:, 2 * P : 3 * P], Op.mult, Op.is_le
            )

            rhs = feats_g[:, j * R : (j + 1) * R]
            st, sp = i == 0, i == N_CHUNKS - 1
            nc.tensor.matmul(ps_s0[:], M[:, 0:CHUNK], rhs, start=st, stop=sp)
            nc.tensor.matmul(ps_s1[:], M[:, CHUNK:P], rhs, start=st, stop=sp)
            nc.tensor.matmul(ps_cnt[:], ones[:], M[:], start=st, stop=sp)

        # ---- allreduce partials ----
        s_sb0 = spool.tile([CHUNK, R], FP)
        s_sb1 = spool.tile([CHUNK, R], FP)
        c_sb = spool.tile([1, P], FP)
        nc.vector.tensor_copy(s_sb0[:], ps_s0[:])
        nc.vector.tensor_copy(s_sb1[:], ps_s1[:])
        nc.vector.tensor_copy(c_sb[:], ps_cnt[:])

        b_in = dram.tile([P + 1, R], FP)
        b_out = dram.tile([P + 1, R], FP)
        nc.gpsimd.dma_start(b_in[0:CHUNK, :], s_sb0[:])
        nc.gpsimd.dma_start(b_in[CHUNK:P, :], s_sb1[:])
        nc.gpsimd.dma_start(b_in[P : P + 1, :], c_sb[:])
        nc.gpsimd.collective_compute(
            "AllReduce", Op.add,
            replica_groups=[list(range(N_CORES))],
            ins=[b_in.opt()], outs=[b_out.opt()],
        )
        rs0 = spool.tile([CHUNK, R], FP)
        rs1 = spool.tile([CHUNK, R], FP)
        rc = spool.tile([1, P], FP)
        nc.gpsimd.dma_start(rs0[:], b_out[0:CHUNK, :])
        nc.gpsimd.dma_start(rs1[:], b_out[CHUNK:P, :])
        nc.gpsimd.dma_start(rc[:], b_out[P : P + 1, :])

        # ---- head (replicated) ----
        # clamp counts, transpose to columns, reciprocal
        cl = spool.tile([1, P], FP)
        nc.vector.tensor_scalar(cl[:], rc[:], 1.0, None, Op.max)
        ps_small = psS.tile([CHUNK, 4 * CO + 2], FP)
        nc.tensor.transpose(ps_small[:, 4 * CO : 4 * CO + 1], cl[:, 0:CHUNK], ident[0:1, 0:1])
        nc.tensor.transpose(ps_small[:, 4 * CO + 1 : 4 * CO + 2], cl[:, CHUNK:P], ident[0:1, 0:1])
        cc0 = spool.tile([CHUNK, 1], FP)
        cc1 = spool.tile([CHUNK, 1], FP)
        nc.vector.tensor_copy(cc0[:], ps_small[:, 4 * CO : 4 * CO + 1])
        nc.vector.tensor_copy(cc1[:], ps_small[:, 4 * CO + 1 : 4 * CO + 2])
        rcp0 = spool.tile([CHUNK, 1], FP)
        rcp1 = spool.tile([CHUNK, 1], FP)
        nc.vector.reciprocal(rcp0[:], cc0[:])
        nc.vector.reciprocal(rcp1[:], cc1[:])

        roi0 = spool.tile([CHUNK, R], FP)
        roi1 = spool.tile([CHUNK, R], FP)
        nc.vector.tensor_scalar(roi0[:], rs0[:], rcp0[:], None, Op.mult)
        nc.vector.tensor_scalar(roi1[:], rs1[:], rcp1[:], None, Op.mult)

        # transpose roi -> roiT [r, p] halves
        ps_rT0 = psB.tile([CHUNK, P], FP)
        ps_rT1 = psB.tile([CHUNK, P], FP)
        nc.tensor.transpose(ps_rT0[:, 0:CHUNK], roi0[:, 0:CHUNK], ident[:])
        nc.tensor.transpose(ps_rT0[:, CHUNK:P], roi1[:, 0:CHUNK], ident[:])
        nc.tensor.transpose(ps_rT1[:, 0:CHUNK], roi0[:, CHUNK:R], ident[:])
        nc.tensor.transpose(ps_rT1[:, CHUNK:P], roi1[:, CHUNK:R], ident[:])
        rT0 = spool.tile([CHUNK, P], FPR)
        rT1 = spool.tile([CHUNK, P], FPR)
        nc.vector.tensor_copy(rT0[:], ps_rT0[:])
        nc.vector.tensor_copy(rT1[:], ps_rT1[:])

        # logitsT [42, 256] = W_cat.T @ roiT + b
        ps_lg = psS.tile([64, P], FP)
        nc.tensor.matmul(ps_lg[:], wc0[:], rT0[:], start=True, stop=False)
        nc.tensor.matmul(ps_lg[:], wc1[:], rT1[:], start=False, stop=True)
        lg = spool.tile([64, P], FP)
        nc.scalar.activation(lg[:], ps_lg[:], Act.Identity, bias=bcat[:], scale=1.0)

        # obj softmax over proposals (free dim) on rows CO..2*CO
        lgo = lg[32 : 32 + CO, :]
        mo = spool.tile([CO, 1], FP)
        nc.vector.tensor_reduce(mo[:], lgo, AxisListType.X, Op.max)
        nmo = spool.tile([CO, 1], FP)
        nc.vector.tensor_scalar(nmo[:], mo[:], -1.0, None, Op.mult)
        eo = spool.tile([CO, P], FP)
        nc.scalar.activation(eo[:], lgo, Act.Exp, bias=nmo[:], scale=1.0)
        so = spool.tile([CO, 1], FP)
        nc.vector.tensor_reduce(so[:], eo[:], AxisListType.X, Op.add)
        ro = spool.tile([CO, 1], FP)
        nc.vector.reciprocal(ro[:], so[:])
        objp = spool.tile([CO, P], FP)
        nc.vector.tensor_scalar(objp[:], eo[:], ro[:], None, Op.mult)

        # cls: transpose logits rows 0..CO -> [p, c] halves, softmax over c (free)
        ps_cT0 = ps_small[:, 0:CO]
        ps_cT1 = ps_small[:, CO : 2 * CO]
        nc.tensor.transpose(ps_cT0, lg[0:CO, 0:CHUNK], ident[0:CO, 0:CO])
        nc.tensor.transpose(ps_cT1, lg[0:CO, CHUNK:P], ident[0:CO, 0:CO])
        # obj_p transposed -> [p, c] halves
        ps_oT0 = ps_small[:, 2 * CO : 3 * CO]
        ps_oT1 = ps_small[:, 3 * CO : 4 * CO]
        nc.tensor.transpose(ps_oT0, objp[:, 0:CHUNK], ident[0:CO, 0:CO])
        nc.tensor.transpose(ps_oT1, objp[:, CHUNK:P], ident[0:CO, 0:CO])

        for h, (ps_cT, ps_oT) in enumerate(((ps_cT0, ps_oT0), (ps_cT1, ps_oT1))):
            ct = spool.tile([CHUNK, CO], FP, tag=f"ct{h}")
            nc.vector.tensor_copy(ct[:], ps_cT)
            mc = spool.tile([CHUNK, 1], FP, tag=f"mc{h}")
            nc.vector.tensor_reduce(mc[:], ct[:], AxisListType.X, Op.max)
            nmc = spool.tile([CHUNK, 1], FP, tag=f"nmc{h}")
            nc.vector.tensor_scalar(nmc[:], mc[:], -1.0, None, Op.mult)
            ec = spool.tile([CHUNK, CO], FP, tag=f"ec{h}")
            nc.scalar.activation(ec[:], ct[:], Act.Exp, bias=nmc[:], scale=1.0)
            sc = spool.tile([CHUNK, 1], FP, tag=f"sc{h}")
            nc.vector.tensor_reduce(sc[:], ec[:], AxisListType.X, Op.add)
            rc2 = spool.tile([CHUNK, 1], FP, tag=f"rc2{h}")
            nc.vector.reciprocal(rc2[:], sc[:])
            clsp = spool.tile([CHUNK, CO], FP, tag=f"clsp{h}")
            nc.vector.tensor_scalar(clsp[:], ec[:], rc2[:], None, Op.mult)
            outh = spool.tile([CHUNK, CO], FP, tag=f"outh{h}")
            nc.vector.tensor_tensor(outh[:], clsp[:], ps_oT, Op.mult)
            nc.gpsimd.dma_start(out_d.ap()[h * CHUNK : (h + 1) * CHUNK, :], outh[:])

    nc.compile()
    return nc


def _round12(a):
    m, e = np.frexp(a.astype(np.float32))
    return (np.round(m * 4096.0) / 4096.0 * np.exp2(e.astype(np.float32))).astype(
        np.float32
    )


def kernel(
    proposals, input_xyz, seg_feats, W_cls, b_cls, W_obj, b_obj, _trace=False
):
    if "nc" not in _cache:
        _cache["nc"] = _build()
    nc = _cache["nc"]

    proposals = np.asarray(proposals, dtype=np.float32)
    input_xyz = np.asarray(input_xyz, dtype=np.float32)
    seg_feats = np.asarray(seg_feats, dtype=np.float32)

    ctr = proposals[:, :3]
    half = proposals[:, 3:] * np.float32(0.5)
    lo = ctr - half
    hi = ctr + half

    bt = np.empty((CHUNK, 6 * P), np.float32)
    for d in range(3):
        bt[:, 2 * d * P : (2 * d + 1) * P] = lo[:, d][None, :]
        bt[:, (2 * d + 1) * P : (2 * d + 2) * P] = hi[:, d][None, :]

    n_tot = N_CORES * N_PAD_CORE
    xyz_pad = np.full((n_tot, 3), 9.0, np.float32)
    xyz_pad[:N] = input_xyz
    feats_pad = np.zeros((n_tot, R), np.float32)
    feats_pad[:N] = seg_feats

    wcat = np.zeros((R, 64), np.float32)
    wcat[:, 0:CO] = W_cls
    wcat[:, 32 : 32 + CO] = W_obj
    bcat = np.zeros((64, 1), np.float32)
    bcat[0:CO, 0] = b_cls
    bcat[32 : 32 + CO, 0] = b_obj
    ident = np.eye(CHUNK, dtype=np.float32)
    ones = np.ones((CHUNK, 1), np.float32)
    c16 = np.zeros((CHUNK, 4), np.int16)
    c16[:, 0] = -7
    c16[:, 1] = -1

    in_maps = []
    for c in range(N_CORES):
        sl = slice(c * N_PAD_CORE, (c + 1) * N_PAD_CORE)
        xyz_c = (
            xyz_pad[sl].reshape(N_CHUNKS, CHUNK, 3).transpose(1, 0, 2)
            .reshape(CHUNK, N_CHUNKS * 3).copy()
        )
        feats_c = (
            feats_pad[sl].reshape(N_CHUNKS, CHUNK, R).transpose(1, 0, 2)
            .reshape(CHUNK, N_CHUNKS * R).copy()
        )
        in_maps.append(
            {
                "feats": feats_c, "xyz": xyz_c, "btiles": bt, "wcat": wcat,
                "bcat": bcat, "ident": ident, "ones": ones, "c16": c16,
            }
        )

    res = run_bass_kernel_spmd(
        nc, in_maps, core_ids=list(range(N_CORES)), trace=_trace
    )
    out = res.results[0]["out"]
    if _trace:
        _cache["last_exec_ns"] = res.exec_time_ns
        _cache["last_results"] = res
    return out
